# revision 1
# baseline (speedup 1.0000x reference)
"""GPT-2 transformer block on 8 Trainium2 NeuronCores.

Sharding: core c = (batch b = c//2, rank r = c%2).  Pairs (2b, 2b+1) share a
batch: each core computes ln1 + qkv for its 6 of 12 heads over the full
sequence (T=2048), causal flash-style attention in transposed layout,
then an intra-pair AllGather of the per-head outputs; aproj + ln2 + FFN run
token-parallel (each core takes its rank's half of the tokens), so no second
collective is needed.  All matmuls run in bf16 with fp32 PSUM accumulation.
LayerNorm gains/biases are folded into the following weights on the host.
"""

import numpy as np
import ml_dtypes

import concourse.bass as bass
import concourse.tile as tile
from concourse import mybir
from concourse.alu_op_type import AluOpType
from concourse.masks import make_identity
from concourse.bass_utils import run_bass_kernel_spmd

BF16 = mybir.dt.bfloat16
F32 = mybir.dt.float32
AF = mybir.ActivationFunctionType

N_EMBED = 768
N_HEAD = 12
HEAD = 64
B, T = 4, 2048
D4 = 4 * N_EMBED          # 3072
HG = N_HEAD // 2          # heads per core = 6
DHG = HG * HEAD           # 384: per-core head dims
TOWN = T // 2             # own tokens per core = 1024
GROUPS = [[2 * i, 2 * i + 1] for i in range(4)]
EPS = 1e-5

# walrus single-wait-per-instruction limit workaround ------------------------


def _split_ctrl_waits(nc, max_waits=1):
    fn = nc.m.functions[0]
    for bb in fn.blocks:
        insts = list(bb.instructions)
        changed = False
        new_list = []
        for inst in insts:
            si = inst.sync_info
            waits = list(si.on_wait) if (si is not None and si.on_wait) else []
            if len(waits) > max_waits:
                keep = waits[-max_waits:]
                extra = waits[:-max_waits]
                k = 0
                while extra:
                    batch, extra = extra[:max_waits], extra[max_waits:]
                    nop = mybir.InstNoOp(name=f"{inst.name}_wsplit{k}", ins=[], outs=[])
                    nop.engine = inst.engine
                    nop.sync_info = mybir.SyncInfo(on_wait=batch, on_update=[])
                    new_list.append(nop)
                    k += 1
                inst.sync_info = mybir.SyncInfo(
                    on_wait=keep, on_update=list(si.on_update) if si.on_update else []
                )
                changed = True
            new_list.append(inst)
        if changed:
            bb.instructions = new_list


# ---------------------------------------------------------------------------
def _layernorm_to_bf16(nc, pools, x_tile, ln_tile, n_sub=3):
    """x_tile [128, 768] f32 -> ln_tile [128, 768] bf16 (normalized, g/b NOT
    applied -- they are folded into downstream weights)."""
    small = pools["small"]
    stats = small.tile([128, n_sub, 6], F32, tag="stats")
    sub = N_EMBED // n_sub
    xv = x_tile.rearrange("p (s d) -> p s d", s=n_sub)
    for s in range(n_sub):
        nc.vector.bn_stats(stats[:, s, :], xv[:, s, :])
    mv = small.tile([128, 2], F32, tag="mv")
    nc.vector.bn_aggr(mv, stats)
    sd = small.tile([128, 1], F32, tag="sd")
    nc.scalar.activation(sd, mv[:, 1:2], AF.Sqrt, bias=pools["eps"], scale=1.0)
    r = small.tile([128, 1], F32, tag="r")
    nc.vector.reciprocal(r, sd)
    nmr = small.tile([128, 1], F32, tag="nmr")
    nc.vector.tensor_mul(nmr, mv[:, 0:1], r)
    nc.vector.tensor_scalar_mul(nmr, nmr, -1.0)
    nc.scalar.activation(ln_tile, x_tile, AF.Identity, bias=nmr, scale=r)


def _transpose_128(nc, pools, src_ap, dst_ap):
    """PE-transpose one [128,128] bf16 block SBUF->SBUF."""
    ps = pools["tpsum"].tile([128, 128], BF16, tag="ps")
    nc.tensor.transpose(ps, src_ap, pools["ident"])
    nc.scalar.copy(dst_ap, ps)


def build_nc():
    nc = bass.Bass()

    x_ext = nc.declare_dram_parameter("x", [T, N_EMBED], F32, isOutput=False)
    wq_ext = nc.declare_dram_parameter("wq", [N_EMBED, DHG], BF16, isOutput=False)
    wk_ext = nc.declare_dram_parameter("wk", [N_EMBED, DHG], BF16, isOutput=False)
    wv_ext = nc.declare_dram_parameter("wv", [N_EMBED, DHG], BF16, isOutput=False)
    bqk_ext = nc.declare_dram_parameter("bqk", [128, 6], F32, isOutput=False)
    bv_ext = nc.declare_dram_parameter("bv", [1, DHG], BF16, isOutput=False)
    wap_ext = nc.declare_dram_parameter("wap", [N_EMBED, N_EMBED], BF16, isOutput=False)
    bap_ext = nc.declare_dram_parameter("bap", [1, N_EMBED], BF16, isOutput=False)
    wfc_ext = nc.declare_dram_parameter("wfc", [N_EMBED, D4], BF16, isOutput=False)
    bfc_ext = nc.declare_dram_parameter("bfc", [128, 24], F32, isOutput=False)
    wmp_ext = nc.declare_dram_parameter("wmp", [D4, N_EMBED], BF16, isOutput=False)
    bmp_ext = nc.declare_dram_parameter("bmp", [1, N_EMBED], BF16, isOutput=False)
    msk_ext = nc.declare_dram_parameter("msk", [128, 4, 1024], BF16, isOutput=False)
    out_ext = nc.declare_dram_parameter("out", [TOWN, N_EMBED], F32, isOutput=True)

    y_bounce = nc.dram_tensor("y_bounce", [4, DHG, 512], BF16)
    ag_bounce = nc.dram_tensor("ag_bounce", [4, 2 * DHG, 512], BF16)

    with tile.TileContext(nc) as tc:
        with (
            tc.tile_pool(name="perm", bufs=1) as perm,
            tc.tile_pool(name="small", bufs=6) as small,
            tc.tile_pool(name="psum", bufs=4, space="PSUM") as psum,
            tc.tile_pool(name="psum2", bufs=2, space="PSUM") as psum2,
            tc.tile_pool(name="xpool", bufs=4) as xpool,
            tc.tile_pool(name="lnp", bufs=3) as lnp,
            tc.tile_pool(name="attp", bufs=6) as att_pool,
        ):
            ident = perm.tile([128, 128], BF16, tag="ident")
            make_identity(nc, ident)
            eps_t = perm.tile([128, 1], F32, tag="eps")
            nc.vector.memset(eps_t, EPS)
            ones_row = perm.tile([1, 128], BF16, tag="ones_row")
            nc.vector.memset(ones_row, 1.0)
            pools = {"small": small, "tpsum": psum, "ident": ident, "eps": eps_t}

            # masks for causal diagonal blocks: [128, 4, 512]
            msk = perm.tile([128, 4, 1024], BF16, tag="msk")
            nc.gpsimd.dma_start(out=msk, in_=msk_ext[:, :, :])

            # resident weights (attention side)
            wq_sb = perm.tile([128, 6, DHG], BF16, tag="wq")
            nc.gpsimd.dma_start(out=wq_sb, in_=wq_ext.rearrange("(c p) m -> p c m", p=128))
            wk_sb = perm.tile([128, 6, DHG], BF16, tag="wk")
            nc.gpsimd.dma_start(out=wk_sb, in_=wk_ext.rearrange("(c p) m -> p c m", p=128))
            wv_sb = perm.tile([128, 6, DHG], BF16, tag="wv")
            nc.gpsimd.dma_start(out=wv_sb, in_=wv_ext.rearrange("(c p) m -> p c m", p=128))
            bqk_sb = perm.tile([128, 6], F32, tag="bqk")
            nc.gpsimd.dma_start(out=bqk_sb, in_=bqk_ext[:, :])
            bv_sb = perm.tile([1, DHG], BF16, tag="bv")
            nc.gpsimd.dma_start(out=bv_sb, in_=bv_ext[:, :])
            wap_sb = perm.tile([128, 6, N_EMBED], BF16, tag="wap")
            nc.gpsimd.dma_start(out=wap_sb, in_=wap_ext.rearrange("(c p) m -> p c m", p=128))
            bap_sb = perm.tile([1, N_EMBED], BF16, tag="bap")
            nc.gpsimd.dma_start(out=bap_sb, in_=bap_ext[:, :])
            bfc_sb = perm.tile([128, 24], F32, tag="bfc")
            nc.gpsimd.dma_start(out=bfc_sb, in_=bfc_ext[:, :])
            bmp_sb = perm.tile([1, N_EMBED], BF16, tag="bmp")
            nc.gpsimd.dma_start(out=bmp_sb, in_=bmp_ext[:, :])

            with tc.tile_pool(name="qkv", bufs=1) as qkv_pool:
                qT = qkv_pool.tile([128, 3, T], BF16, tag="qT")
                kT = qkv_pool.tile([128, 3, T], BF16, tag="kT")
                v_sb = qkv_pool.tile([128, 16, HG, 2 * HEAD], BF16, tag="v_sb")
                nc.vector.memset(v_sb[:, :, :, HEAD : 2 * HEAD], 1.0)

                # ========== phase A: ln1 over full T + transpose ==========
                with tc.tile_pool(name="lnT", bufs=1) as lnT_pool:
                    ln1xT = lnT_pool.tile([128, 6, T], BF16, tag="ln1xT")
                    for t in range(16):
                        x_t = xpool.tile([128, N_EMBED], F32, tag="x_t")
                        nc.sync.dma_start(out=x_t, in_=x_ext[128 * t : 128 * (t + 1), :])
                        ln_t = lnp.tile([128, N_EMBED], BF16, tag="ln_t")
                        _layernorm_to_bf16(nc, pools, x_t, ln_t)
                        for c in range(6):
                            _transpose_128(
                                nc, pools, ln_t[:, 128 * c : 128 * (c + 1)],
                                ln1xT[:, c, 128 * t : 128 * (t + 1)],
                            )

                    # ========== phase B: Q^T, K^T, V ==========
                    for dst, w_sb, bias_col in ((qT, wq_sb, 0), (kT, wk_sb, 3)):
                        for m in range(3):
                            for t4 in range(4):
                                ps = psum.tile([128, 512], F32, tag="ps")
                                for c in range(6):
                                    nc.tensor.matmul(
                                        ps,
                                        lhsT=w_sb[:, c, 128 * m : 128 * (m + 1)],
                                        rhs=ln1xT[:, c, 512 * t4 : 512 * (t4 + 1)],
                                        start=(c == 0),
                                        stop=(c == 5),
                                    )
                                nc.scalar.activation(
                                    dst[:, m, 512 * t4 : 512 * (t4 + 1)], ps, AF.Identity,
                                    bias=bqk_sb[:, bias_col + m : bias_col + m + 1], scale=1.0,
                                )
                    for t in range(16):
                        ps = psum.tile([128, 512], F32, tag="ps")
                        for c in range(6):
                            nc.tensor.matmul(
                                ps[:, 0:DHG],
                                lhsT=ln1xT[:, c, 128 * t : 128 * (t + 1)],
                                rhs=wv_sb[:, c, :],
                                start=(c == 0),
                                stop=False,
                            )
                        nc.tensor.matmul(
                            ps[:, 0:DHG], lhsT=ones_row, rhs=bv_sb, start=False, stop=True
                        )
                        nc.vector.tensor_copy(
                            v_sb[:, t, :, 0:HEAD],
                            ps[:, 0:DHG].rearrange("p (h d) -> p h d", h=HG),
                        )

                # ========== phase C: attention ==========
                with tc.tile_pool(name="yTp", bufs=1) as yT_pool:
                    yT = yT_pool.tile([128, 3, T], BF16, tag="yT")
                    for qc in range(4):
                        qoff = 512 * qc
                        nkb = 4 * (qc + 1)
                        for hp in range(3):
                            ps_y = [
                                psum.tile([128, 512], F32, tag="ps", name=f"psy0_{qc}_{hp}"),
                                psum.tile([128, 512], F32, tag="ps", name=f"psy1_{qc}_{hp}"),
                            ]
                            for kb in range(nkb):
                                ps_s = psum2.tile([128, 1024], F32, tag="ps2")
                                for h2 in range(2):
                                    lo, hi = 64 * h2, 64 * (h2 + 1)
                                    nc.tensor.matmul(
                                        ps_s[:, 512 * h2 : 512 * (h2 + 1)],
                                        lhsT=kT[lo:hi, hp, 128 * kb : 128 * (kb + 1)],
                                        rhs=qT[lo:hi, hp, qoff : qoff + 512],
                                        start=True,
                                        stop=True,
                                    )
                                att = att_pool.tile([128, 1024], BF16, tag="att")
                                nc.scalar.activation(att, ps_s, AF.Exp)
                                j = kb - 4 * qc
                                if j >= 0:
                                    w = 128 * (j + 1)
                                    for h2 in range(2):
                                        nc.vector.tensor_mul(
                                            att[:, 512 * h2 : 512 * h2 + w],
                                            att[:, 512 * h2 : 512 * h2 + w],
                                            msk[:, j, 0:w],
                                        )
                                for h2 in range(2):
                                    nc.tensor.matmul(
                                        ps_y[h2],
                                        lhsT=v_sb[:, kb, 2 * hp + h2, :],
                                        rhs=att[:, 512 * h2 : 512 * (h2 + 1)],
                                        start=(kb == 0),
                                        stop=(kb == nkb - 1),
                                        skip_group_check=True,
                                    )
                            for h2 in range(2):
                                rec_bc = att_pool.tile([HEAD, 512], F32, tag="rec_bc")
                                nc.vector.reciprocal(
                                    rec_bc, ps_y[h2][HEAD : 2 * HEAD, :]
                                )
                                nc.vector.tensor_mul(
                                    yT[64 * h2 : 64 * (h2 + 1), hp, qoff : qoff + 512],
                                    ps_y[h2][0:HEAD, :],
                                    rec_bc,
                                )

                        # AllGather each 512-token quarter as soon as it is done
                        nc.sync.dma_start(
                            out=y_bounce[qc].rearrange("(c p) n -> p c n", p=128),
                            in_=yT[:, :, 512 * qc : 512 * (qc + 1)],
                        )
                        nc.gpsimd.collective_compute(
                            "AllGather",
                            AluOpType.bypass,
                            replica_groups=GROUPS,
                            ins=[y_bounce[qc][:]],
                            outs=[ag_bounce[qc][:]],
                        )

            with tc.tile_pool(name="x1p", bufs=1) as x1_pool:
                x1 = x1_pool.tile([128, 8, N_EMBED], F32, tag="x1")
                with (
                    tc.tile_pool(name="xo", bufs=1) as xo_pool,
                    tc.tile_pool(name="yf", bufs=1) as yf_pool,
                ):
                    # rank (0/1 within pair) for dynamic token-half addressing
                    rank_reg = nc.gpsimd.alloc_register()
                    nc.gpsimd.cc_rank_ld(rank_reg, replica_groups=GROUPS)
                    rank = nc.gpsimd.snap(rank_reg, donate=True)

                    x_own = xo_pool.tile([128, 8, N_EMBED], F32, tag="x_own")
                    x_halves = x_ext.rearrange("(h n p) d -> p h n d", h=2, p=128)
                    nc.gpsimd.dma_start(out=x_own, in_=x_halves[:, bass.ds(rank, 1), :, :])
                    yfT = yf_pool.tile([128, 2, 6, 512], BF16, tag="yfT")
                    ag_v = ag_bounce.rearrange("(h q) (c p) n -> p h q c n", h=2, p=128)
                    nc.gpsimd.dma_start(out=yfT, in_=ag_v[:, bass.ds(rank, 1), :, :, :])

                    # ========== phase E: aproj + residual ==========
                    for t in range(8):
                        for n0, n1 in ((0, 512), (512, 768)):
                            ps = psum.tile([128, 512], F32, tag="ps")
                            w = n1 - n0
                            for c in range(6):
                                nc.tensor.matmul(
                                    ps[:, 0:w],
                                    lhsT=yfT[:, t // 4, c, 128 * (t % 4) : 128 * (t % 4 + 1)],
                                    rhs=wap_sb[:, c, n0:n1],
                                    start=(c == 0),
                                    stop=False,
                                )
                            nc.tensor.matmul(
                                ps[:, 0:w], lhsT=ones_row, rhs=bap_sb[:, n0:n1],
                                start=False, stop=True,
                            )
                            nc.vector.tensor_add(
                                x1[:, t, n0:n1], ps[:, 0:w], x_own[:, t, n0:n1]
                            )

                with tc.tile_pool(name="hTp", bufs=1) as hT_pool:
                    hT = hT_pool.tile([128, 24, TOWN], BF16, tag="hT")
                    # ========== phase F: ln2 + transpose ==========
                    with tc.tile_pool(name="ln2T", bufs=1) as ln2T_pool:
                        ln2xT = ln2T_pool.tile([128, 6, TOWN], BF16, tag="ln2xT")
                        for t in range(8):
                            ln_t = lnp.tile([128, N_EMBED], BF16, tag="ln_t")
                            _layernorm_to_bf16(nc, pools, x1[:, t, :], ln_t)
                            for c in range(6):
                                _transpose_128(
                                    nc, pools, ln_t[:, 128 * c : 128 * (c + 1)],
                                    ln2xT[:, c, 128 * t : 128 * (t + 1)],
                                )

                        # ========== phase G: fc + gelu ==========
                        with tc.tile_pool(name="wfc", bufs=1) as wfc_pool:
                            wfc_sb = wfc_pool.tile([128, 6, D4], BF16, tag="wfc")
                            nc.sync.dma_start(
                                out=wfc_sb, in_=wfc_ext.rearrange("(c p) m -> p c m", p=128)
                            )
                            for m in range(24):
                                for t2 in range(2):
                                    ps = psum.tile([128, 512], F32, tag="ps")
                                    for c in range(6):
                                        nc.tensor.matmul(
                                            ps,
                                            lhsT=wfc_sb[:, c, 128 * m : 128 * (m + 1)],
                                            rhs=ln2xT[:, c, 512 * t2 : 512 * (t2 + 1)],
                                            start=(c == 0),
                                            stop=(c == 5),
                                        )
                                    nc.scalar.activation(
                                        hT[:, m, 512 * t2 : 512 * (t2 + 1)], ps, AF.Gelu,
                                        bias=bfc_sb[:, m : m + 1], scale=1.0,
                                    )

                    # ========== phase H: mproj + residual + out ==========
                    with (
                        tc.tile_pool(name="wmp", bufs=24) as wmp_pool,
                        tc.tile_pool(name="outp", bufs=3) as outp,
                    ):
                        wmp_tiles = []
                        for hc in range(24):
                            wt = wmp_pool.tile([128, N_EMBED], BF16, tag="wmp", name=f"wmp{hc}")
                            nc.sync.dma_start(
                                out=wt, in_=wmp_ext[128 * hc : 128 * (hc + 1), :]
                            )
                            wmp_tiles.append(wt)
                        for t in range(8):
                            o_t = outp.tile([128, N_EMBED], F32, tag="o_t")
                            for n0, n1 in ((0, 512), (512, 768)):
                                ps = psum.tile([128, 512], F32, tag="ps")
                                w = n1 - n0
                                for hc in range(24):
                                    nc.tensor.matmul(
                                        ps[:, 0:w],
                                        lhsT=hT[:, hc, 128 * t : 128 * (t + 1)],
                                        rhs=wmp_tiles[hc][:, n0:n1],
                                        start=(hc == 0),
                                        stop=False,
                                    )
                                nc.tensor.matmul(
                                    ps[:, 0:w], lhsT=ones_row, rhs=bmp_sb[:, n0:n1],
                                    start=False, stop=True,
                                )
                                nc.vector.tensor_add(o_t[:, n0:n1], ps[:, 0:w], x1[:, t, n0:n1])
                            nc.sync.dma_start(
                                out=out_ext[128 * t : 128 * (t + 1), :], in_=o_t
                            )

    _split_ctrl_waits(nc)
    return nc


_NC_CACHE = None


def _get_nc():
    global _NC_CACHE
    if _NC_CACHE is None:
        _NC_CACHE = build_nc()
    return _NC_CACHE


def _prep_inputs(x, ln1_g, ln1_b, w_attn, b_attn, w_aproj, b_aproj,
                 ln2_g, ln2_b, w_fc, b_fc, w_mproj, b_mproj):
    bf = ml_dtypes.bfloat16
    f32 = np.float32
    x = np.asarray(x, f32)
    ln1_g = np.asarray(ln1_g, f32); ln1_b = np.asarray(ln1_b, f32)
    ln2_g = np.asarray(ln2_g, f32); ln2_b = np.asarray(ln2_b, f32)
    w_attn = np.asarray(w_attn, f32); b_attn = np.asarray(b_attn, f32)
    w_aproj = np.asarray(w_aproj, f32); b_aproj = np.asarray(b_aproj, f32)
    w_fc = np.asarray(w_fc, f32); b_fc = np.asarray(b_fc, f32)
    w_mproj = np.asarray(w_mproj, f32); b_mproj = np.asarray(b_mproj, f32)

    # fold ln1 gain into w_attn rows; ln1 bias into b_attn
    w_attn_f = ln1_g[:, None] * w_attn
    b_attn_f = b_attn + ln1_b @ w_attn
    wq = w_attn_f[:, 0:N_EMBED]; bq = b_attn_f[0:N_EMBED]
    wk = w_attn_f[:, N_EMBED : 2 * N_EMBED]; bk = b_attn_f[N_EMBED : 2 * N_EMBED]
    wv = w_attn_f[:, 2 * N_EMBED :]; bv = b_attn_f[2 * N_EMBED :]
    scale = 1.0 / np.sqrt(HEAD)
    wq = wq * scale; bq = bq * scale

    w_fc_f = ln2_g[:, None] * w_fc
    b_fc_f = b_fc + ln2_b @ w_fc

    # causal diagonal masks in transposed layout: msk[k, j, q] = k + 128j <= q
    kk = np.arange(128)[:, None, None]
    jj = np.arange(4)[None, :, None]
    qq = np.arange(512)[None, None, :]
    msk = ((kk + 128 * jj) <= qq).astype(bf)
    msk = np.concatenate([msk, msk], axis=2)  # two heads side by side

    per_rank = []
    for r in range(2):
        hsel = slice(r * DHG, (r + 1) * DHG)  # this rank's 6 heads (x64 dims)
        bqk = np.zeros((128, 6), f32)
        for m in range(3):
            bqk[:, m] = bq[hsel][128 * m : 128 * (m + 1)]
            bqk[:, 3 + m] = bk[hsel][128 * m : 128 * (m + 1)]
        per_rank.append(
            dict(
                wq=np.ascontiguousarray(wq[:, hsel]).astype(bf),
                wk=np.ascontiguousarray(wk[:, hsel]).astype(bf),
                wv=np.ascontiguousarray(wv[:, hsel]).astype(bf),
                bqk=bqk,
                bv=np.ascontiguousarray(bv[hsel])[None, :].astype(bf),
                wap=w_aproj.astype(bf),
                bap=b_aproj[None, :].astype(bf),
                wfc=w_fc_f.astype(bf),
                bfc=np.ascontiguousarray(
                    b_fc_f.reshape(24, 128).T
                ).astype(f32),
                wmp=w_mproj.astype(bf),
                bmp=b_mproj[None, :].astype(bf),
                msk=msk,
            )
        )

    in_maps = []
    for c in range(8):
        b_i, r = c // 2, c % 2
        m = dict(per_rank[r])
        m["x"] = np.ascontiguousarray(x[b_i])
        in_maps.append(m)
    return in_maps


def kernel(**inputs):
    nc = _get_nc()
    in_maps = _prep_inputs(**inputs)
    res = run_bass_kernel_spmd(nc, in_maps, list(range(8)))
    out = np.empty((B, T, N_EMBED), np.float32)
    for c in range(8):
        b_i, r = c // 2, c % 2
        out[b_i, r * TOWN : (r + 1) * TOWN, :] = res.results[c]["out"]
    return out



# revision 10
# speedup vs baseline: 1.1223x; 1.1223x over previous
"""GPT-2 transformer block on 8 Trainium2 NeuronCores.

Sharding: core c = (batch b = c//2, rank r = c%2).  Pairs (2b, 2b+1) share a
batch: each core computes ln1 + qkv for its 6 of 12 heads over the full
sequence (T=2048), causal attention in transposed layout, an intra-pair
AllGather of per-head outputs per 512-token quarter, then token-parallel
aproj + ln2 + FFN where rank r owns quarters {r, r+2}.  The FFN work for the
first owned quarter is interleaved into the scalar-bound (exp) attention
stream of quarters 2-3 so the PE array never idles.  Dense matmuls (qkv,
aproj, fc, mproj) run in fp8-e4m3 DoubleRow (pairs of adjacent 128-chunks
along the contraction dim); attention scores/AV stay bf16.  Weights are
pre-scaled by 64 before the fp8 cast (avoids the subnormal range) and the
1/64 is folded into the consuming activation/vector op; the attention score
scale 1/sqrt(64) is folded into the exp activation's scale.  LayerNorm
gains/biases and b_aproj are folded into weights / the residual input on the
host.
"""

import numpy as np
import ml_dtypes

import concourse.bass as bass
import concourse.tile as tile
from concourse import mybir
from concourse.alu_op_type import AluOpType
from concourse.masks import make_identity
from concourse.bass_utils import run_bass_kernel_spmd

BF16 = mybir.dt.bfloat16
F32 = mybir.dt.float32
F8 = mybir.dt.float8e4
AF = mybir.ActivationFunctionType
DR = mybir.MatmulPerfMode.DoubleRow
MUL = AluOpType.mult
ADD = AluOpType.add

N_EMBED = 768
N_HEAD = 12
HEAD = 64
B, T = 4, 2048
D4 = 4 * N_EMBED          # 3072
HG = N_HEAD // 2          # heads per core = 6
DHG = HG * HEAD           # 384
TOWN = T // 2             # own tokens per core = 1024
GROUPS = [[2 * i, 2 * i + 1] for i in range(4)]
EPS = 1e-5
WS = 64.0                 # fp8 weight pre-scale
IWS = 1.0 / WS

# walrus single-wait-per-instruction limit workaround ------------------------


def _split_ctrl_waits(nc, max_waits=1):
    fn = nc.m.functions[0]
    for bb in fn.blocks:
        insts = list(bb.instructions)
        changed = False
        new_list = []
        for inst in insts:
            si = inst.sync_info
            waits = list(si.on_wait) if (si is not None and si.on_wait) else []
            if len(waits) > max_waits:
                keep = waits[-max_waits:]
                extra = waits[:-max_waits]
                k = 0
                while extra:
                    batch, extra = extra[:max_waits], extra[max_waits:]
                    nop = mybir.InstNoOp(name=f"{inst.name}_wsplit{k}", ins=[], outs=[])
                    nop.engine = inst.engine
                    nop.sync_info = mybir.SyncInfo(on_wait=batch, on_update=[])
                    new_list.append(nop)
                    k += 1
                inst.sync_info = mybir.SyncInfo(
                    on_wait=keep, on_update=list(si.on_update) if si.on_update else []
                )
                changed = True
            new_list.append(inst)
        if changed:
            bb.instructions = new_list


# ---------------------------------------------------------------------------


def _ln_stats4(nc, pools, x_aps):
    """Batched LN stats over 4 [128,768] f32 tiles -> (r, nmr) [128,4] f32."""
    small = pools["small"]
    n = len(x_aps)
    stats = small.tile([128, n, 3, 6], F32, tag="stats")
    for t, x in enumerate(x_aps):
        xv = x.rearrange("p (s d) -> p s d", s=3)
        for s in range(3):
            nc.vector.bn_stats(stats[:, t, s, :], xv[:, s, :])
    mv = small.tile([128, n, 2], F32, tag="mv")
    for t in range(n):
        nc.vector.bn_aggr(mv[:, t, :], stats[:, t, :, :])
    sd = small.tile([128, n], F32, tag="sd")
    nc.scalar.activation(sd, mv[:, :, 1], AF.Sqrt, bias=pools["eps"], scale=1.0)
    r = small.tile([128, n], F32, tag="r")
    nc.vector.reciprocal(r, sd)
    nmr = small.tile([128, n], F32, tag="nmr")
    nc.vector.scalar_tensor_tensor(nmr, mv[:, :, 0], -1.0, r, op0=MUL, op1=MUL)
    return r, nmr


def _ln_apply_transpose(nc, pools, x_ap, r1, nmr1, dstT, tcol, on_vector):
    """Normalize x [128,768] -> fp8, PE-transpose into dstT[:, c, tcol:+128]."""
    ln_t = pools["lnp"].tile([128, N_EMBED], BF16, tag="ln_t")
    nc.vector.tensor_scalar(ln_t, x_ap, r1, nmr1, op0=MUL, op1=ADD)
    psf = pools["psum_f"]
    ps4 = psf.tile([128, 512], BF16, tag="ps", name="tp4")
    for c in range(4):
        nc.tensor.transpose(
            ps4[:, 128 * c : 128 * (c + 1)], ln_t[:, 128 * c : 128 * (c + 1)],
            pools["ident"],
        )
    ps2 = psf.tile([128, 512], BF16, tag="ps", name="tp2")
    for c in range(2):
        nc.tensor.transpose(
            ps2[:, 128 * c : 128 * (c + 1)], ln_t[:, 512 + 128 * c : 512 + 128 * (c + 1)],
            pools["ident"],
        )
    src4 = ps4.rearrange("p (c x) -> p c x", c=4)
    src2 = ps2[:, 0:256].rearrange("p (c x) -> p c x", c=2)
    if on_vector:
        nc.vector.tensor_copy(dstT[:, 0:4, tcol : tcol + 128], src4)
        nc.vector.tensor_copy(dstT[:, 4:6, tcol : tcol + 128], src2)
    else:
        nc.scalar.copy(dstT[:, 0:4, tcol : tcol + 128], src4)
        nc.scalar.copy(dstT[:, 4:6, tcol : tcol + 128], src2)


def build_nc():
    nc = bass.Bass()

    x_ext = nc.declare_dram_parameter("x", [T, N_EMBED], F32, isOutput=False)
    xb_ext = nc.declare_dram_parameter("xb", [T, N_EMBED], F32, isOutput=False)
    wq_ext = nc.declare_dram_parameter("wq", [128, 3, 2, DHG], F8, isOutput=False)
    wk_ext = nc.declare_dram_parameter("wk", [128, 3, 2, DHG], F8, isOutput=False)
    wv_ext = nc.declare_dram_parameter("wv", [128, 3, 2, DHG], F8, isOutput=False)
    bqk_ext = nc.declare_dram_parameter("bqk", [128, 6], F32, isOutput=False)
    bv_ext = nc.declare_dram_parameter("bv", [1, DHG], BF16, isOutput=False)
    wap_ext = nc.declare_dram_parameter("wap", [128, 3, 2, N_EMBED], F8, isOutput=False)
    wfc_ext = nc.declare_dram_parameter("wfc", [128, 3, 2, D4], F8, isOutput=False)
    bfc_ext = nc.declare_dram_parameter("bfc", [128, 24], F32, isOutput=False)
    wmp_ext = nc.declare_dram_parameter("wmp", [128, 12, 2, N_EMBED], F8, isOutput=False)
    bmp_ext = nc.declare_dram_parameter("bmp", [1, N_EMBED], BF16, isOutput=False)
    msk_ext = nc.declare_dram_parameter("msk", [128, 4, 512], BF16, isOutput=False)
    out_ext = nc.declare_dram_parameter("out", [TOWN, N_EMBED], F32, isOutput=True)

    y_bounce = nc.dram_tensor("y_bounce", [4, DHG, 512], F8)
    ag_bounce = nc.dram_tensor("ag_bounce", [4, 2 * DHG, 512], F8)

    with tile.TileContext(nc) as tc:
        with (
            tc.tile_pool(name="perm", bufs=1) as perm,
            tc.tile_pool(name="small", bufs=3) as small,
            tc.tile_pool(name="psum_s", bufs=2, space="PSUM") as psum_s,
            tc.tile_pool(name="psum_y", bufs=2, space="PSUM") as psum_y,
            tc.tile_pool(name="psum_f", bufs=2, space="PSUM") as psum_f,
            tc.tile_pool(name="lnp", bufs=3) as lnp,
        ):
            # rank within the pair, for dynamic token-quarter addressing
            rank_reg = nc.gpsimd.alloc_register()
            nc.gpsimd.cc_rank_ld(rank_reg, replica_groups=GROUPS)
            rank = nc.gpsimd.snap(rank_reg, donate=True)

            ident = perm.tile([128, 128], BF16, tag="ident")
            make_identity(nc, ident)
            eps_t = perm.tile([128, 1], F32, tag="eps")
            nc.vector.memset(eps_t, EPS)
            ones_row = perm.tile([1, 128], BF16, tag="ones_row")
            nc.vector.memset(ones_row, 1.0)

            # PE warm-up: keep the array busy while the first DMAs land so
            # the HAM clock gate opens before the real matmuls start.
            for wix in range(10):
                pw = psum_s.tile([128, 1024], F32, tag="ps2", name=f"warm{wix}")
                nc.tensor.matmul(pw[:, 0:128], lhsT=ident, rhs=ident, start=True, stop=True)

            msk = perm.tile([128, 4, 512], BF16, tag="msk")
            nc.gpsimd.dma_start(out=msk, in_=msk_ext[:, :, :])
            wq_sb = perm.tile([128, 3, 2, DHG], F8, tag="wq")
            nc.gpsimd.dma_start(out=wq_sb, in_=wq_ext[:, :, :, :])
            wk_sb = perm.tile([128, 3, 2, DHG], F8, tag="wk")
            nc.gpsimd.dma_start(out=wk_sb, in_=wk_ext[:, :, :, :])
            wv_sb = perm.tile([128, 3, 2, DHG], F8, tag="wv")
            nc.gpsimd.dma_start(out=wv_sb, in_=wv_ext[:, :, :, :])
            bqk_sb = perm.tile([128, 6], F32, tag="bqk")
            nc.gpsimd.dma_start(out=bqk_sb, in_=bqk_ext[:, :])
            bv_sb = perm.tile([1, DHG], BF16, tag="bv")
            nc.gpsimd.dma_start(out=bv_sb, in_=bv_ext[:, :])
            wap_sb = perm.tile([128, 3, 2, N_EMBED], F8, tag="wap")
            nc.gpsimd.dma_start(out=wap_sb, in_=wap_ext[:, :, :, :])
            wfc_sb = perm.tile([128, 3, 2, D4], F8, tag="wfc")
            nc.gpsimd.dma_start(out=wfc_sb, in_=wfc_ext[:, :, :, :])
            bfc_sb = perm.tile([128, 24], F32, tag="bfc")
            nc.gpsimd.dma_start(out=bfc_sb, in_=bfc_ext[:, :])
            wmp_sb = perm.tile([128, 12, 2, N_EMBED], F8, tag="wmp")
            nc.gpsimd.dma_start(out=wmp_sb, in_=wmp_ext[:, :, :, :])
            bmp_sb = perm.tile([1, N_EMBED], BF16, tag="bmp")
            nc.gpsimd.dma_start(out=bmp_sb, in_=bmp_ext[:, :])

            pools = {
                "small": small, "psum_f": psum_f, "lnp": lnp,
                "ident": ident, "eps": eps_t,
            }

            with tc.tile_pool(name="qkv", bufs=1) as qkv_pool:
                qT = qkv_pool.tile([128, 3, T], BF16, tag="qT")
                kT = qkv_pool.tile([128, 3, T], BF16, tag="kT")
                v_sb = qkv_pool.tile([128, 16, HG, 2 * HEAD], BF16, tag="v_sb")
                nc.vector.memset(v_sb[:, :, :, HEAD : 2 * HEAD], 1.0)

                # ===== phase A+B: ln1, transpose, qkv (interleaved per 512) =====
                with (
                    tc.tile_pool(name="xpool", bufs=8) as xpool,
                    tc.tile_pool(name="lnT", bufs=1) as lnT_pool,
                ):
                    ln1xT = lnT_pool.tile([128, 6, T], F8, tag="ln1xT")
                    for g in range(4):
                        xts = []
                        for tl in range(4):
                            t = 4 * g + tl
                            x_t = xpool.tile([128, N_EMBED], F32, tag="x_t", name=f"x{t}")
                            nc.sync.dma_start(out=x_t, in_=x_ext[128 * t : 128 * (t + 1), :])
                            xts.append(x_t)
                        r4, nmr4 = _ln_stats4(nc, pools, xts)
                        for tl in range(4):
                            _ln_apply_transpose(
                                nc, pools, xts[tl], r4[:, tl : tl + 1],
                                nmr4[:, tl : tl + 1], ln1xT, 128 * (4 * g + tl),
                                on_vector=False,
                            )
                        # QK for this 512-token quarter
                        for dst, w_sb, bcol in ((qT, wq_sb, 0), (kT, wk_sb, 3)):
                            for m in range(3):
                                ps = psum_f.tile([128, 512], F32, tag="ps", name=f"qk{g}{m}")
                                for a in range(3):
                                    nc.tensor.matmul(
                                        ps,
                                        lhsT=w_sb[:, a, :, 128 * m : 128 * (m + 1)],
                                        rhs=ln1xT[:, 2 * a : 2 * a + 2, 512 * g : 512 * (g + 1)],
                                        start=(a == 0), stop=(a == 2), perf_mode=DR,
                                    )
                                nc.scalar.activation(
                                    dst[:, m, 512 * g : 512 * (g + 1)], ps, AF.Identity,
                                    bias=bqk_sb[:, bcol + m : bcol + m + 1], scale=IWS,
                                )
                        # V for these 4 token tiles
                        for tl in range(4):
                            t = 4 * g + tl
                            ps = psum_f.tile([128, 512], F32, tag="ps", name=f"v{t}")
                            for a in range(3):
                                nc.tensor.matmul(
                                    ps[:, 0:DHG],
                                    lhsT=ln1xT[:, 2 * a : 2 * a + 2, 128 * t : 128 * (t + 1)],
                                    rhs=wv_sb[:, a, :, :],
                                    start=(a == 0), stop=False, perf_mode=DR,
                                )
                            nc.tensor.matmul(
                                ps[:, 0:DHG], lhsT=ones_row, rhs=bv_sb,
                                start=False, stop=True, skip_group_check=True,
                            )
                            nc.vector.tensor_scalar(
                                v_sb[:, t, :, 0:HEAD],
                                ps[:, 0:DHG].rearrange("p (h d) -> p h d", h=HG),
                                IWS, None, op0=MUL,
                            )

                # ===== phase C: attention + interleaved FFN =====
                with (
                    tc.tile_pool(name="attp", bufs=4) as att_pool,
                    tc.tile_pool(name="yTp", bufs=2) as yT_pool,
                    tc.tile_pool(name="yfp", bufs=2) as yf_pool,
                    tc.tile_pool(name="xbp", bufs=4) as xb_pool,
                    tc.tile_pool(name="x1p", bufs=1) as x1_pool,
                    tc.tile_pool(name="ln2p", bufs=2) as ln2_pool,
                    tc.tile_pool(name="hTp", bufs=1) as hT_pool,
                    tc.tile_pool(name="outp", bufs=3) as outp,
                ):
                    xb_view = xb_ext.rearrange(
                        "(ii hh n p) d -> p ii hh n d", ii=2, hh=2, n=4, p=128
                    )
                    ag_v = ag_bounce.rearrange(
                        "(ii hh) (c p) n -> p ii hh c n", ii=2, hh=2, p=128
                    )

                    def ffn_block(i):
                        # processes token quarter (2*i + rank): 4 tiles of 128
                        yfT = yf_pool.tile([128, HG, 512], F8, tag="yf", name=f"yf{i}")
                        nc.gpsimd.dma_start(
                            out=yfT, in_=ag_v[:, i, bass.ds(rank, 1), :, :]
                        )
                        xbts = []
                        for tt in range(4):
                            xbt = xb_pool.tile(
                                [128, N_EMBED], F32, tag="xb", name=f"xb{i}{tt}"
                            )
                            nc.gpsimd.dma_start(
                                out=xbt, in_=xb_view[:, i, bass.ds(rank, 1), tt, :]
                            )
                            xbts.append(xbt)
                        yield  # DMAs issued

                        x1 = x1_pool.tile([128, 4, N_EMBED], F32, tag="x1", name=f"x1_{i}")
                        for tt in range(4):
                            for n0, n1 in ((0, 512), (512, 768)):
                                w = n1 - n0
                                ps = psum_f.tile(
                                    [128, 512], F32, tag="ps", name=f"ap{i}{tt}{n0}"
                                )
                                for a in range(3):
                                    nc.tensor.matmul(
                                        ps[:, 0:w],
                                        lhsT=yfT[:, 2 * a : 2 * a + 2, 128 * tt : 128 * (tt + 1)],
                                        rhs=wap_sb[:, a, :, n0:n1],
                                        start=(a == 0), stop=(a == 2), perf_mode=DR,
                                    )
                                nc.vector.scalar_tensor_tensor(
                                    x1[:, tt, n0:n1], ps[:, 0:w], IWS, xbts[tt][:, n0:n1],
                                    op0=MUL, op1=ADD,
                                )
                        yield  # aproj done

                        r4, nmr4 = _ln_stats4(nc, pools, [x1[:, tt, :] for tt in range(4)])
                        ln2xT = ln2_pool.tile([128, 6, 512], F8, tag="ln2xT", name=f"l2T{i}")
                        for tt in range(4):
                            _ln_apply_transpose(
                                nc, pools, x1[:, tt, :], r4[:, tt : tt + 1],
                                nmr4[:, tt : tt + 1], ln2xT, 128 * tt, on_vector=True,
                            )
                        yield  # ln2 done

                        hT = hT_pool.tile([128, 24, 512], F8, tag="hT", name=f"hT{i}")
                        for m in range(24):
                            ps = psum_f.tile([128, 512], F32, tag="ps", name=f"fc{i}{m}")
                            for a in range(3):
                                nc.tensor.matmul(
                                    ps,
                                    lhsT=wfc_sb[:, a, :, 128 * m : 128 * (m + 1)],
                                    rhs=ln2xT[:, 2 * a : 2 * a + 2, :],
                                    start=(a == 0), stop=(a == 2), perf_mode=DR,
                                )
                            nc.scalar.activation(
                                hT[:, m, :], ps, AF.Gelu,
                                bias=bfc_sb[:, m : m + 1], scale=IWS,
                            )
                        yield  # fc done

                        for tt in range(4):
                            o_t = outp.tile([128, N_EMBED], F32, tag="o_t", name=f"o{i}{tt}")
                            for n0, n1 in ((0, 512), (512, 768)):
                                w = n1 - n0
                                ps = psum_f.tile(
                                    [128, 512], F32, tag="ps", name=f"mp{i}{tt}{n0}"
                                )
                                for a in range(12):
                                    nc.tensor.matmul(
                                        ps[:, 0:w],
                                        lhsT=hT[:, 2 * a : 2 * a + 2, 128 * tt : 128 * (tt + 1)],
                                        rhs=wmp_sb[:, a, :, n0:n1],
                                        start=(a == 0), stop=False, perf_mode=DR,
                                    )
                                nc.tensor.matmul(
                                    ps[:, 0:w], lhsT=ones_row, rhs=bmp_sb[:, n0:n1],
                                    start=False, stop=True, skip_group_check=True,
                                )
                                nc.vector.scalar_tensor_tensor(
                                    o_t[:, n0:n1], ps[:, 0:w], IWS, x1[:, tt, n0:n1],
                                    op0=MUL, op1=ADD,
                                )
                            nc.sync.dma_start(
                                out=out_ext[512 * i + 128 * tt : 512 * i + 128 * (tt + 1), :],
                                in_=o_t,
                            )
                            if tt == 1:
                                yield  # first half of mproj done

                    def attn_hp(qc, hp, yTq):
                        qoff = 512 * qc
                        nkb = 4 * (qc + 1)
                        ps_y = [
                            psum_y.tile([128, 512], F32, tag="py", name=f"py{qc}{hp}{h2}")
                            for h2 in range(2)
                        ]
                        for kb in range(nkb):
                            ps_s = psum_s.tile([128, 1024], F32, tag="ps2")
                            for h2 in range(2):
                                lo, hi = 64 * h2, 64 * (h2 + 1)
                                nc.tensor.matmul(
                                    ps_s[:, 512 * h2 : 512 * (h2 + 1)],
                                    lhsT=kT[lo:hi, hp, 128 * kb : 128 * (kb + 1)],
                                    rhs=qT[lo:hi, hp, qoff : qoff + 512],
                                    start=True, stop=True,
                                )
                            att = att_pool.tile([128, 1024], BF16, tag="att")
                            nc.scalar.activation(att, ps_s, AF.Exp, scale=0.125)
                            j = kb - 4 * qc
                            if j >= 0:
                                w = 128 * (j + 1)
                                for h2 in range(2):
                                    nc.vector.tensor_mul(
                                        att[:, 512 * h2 : 512 * h2 + w],
                                        att[:, 512 * h2 : 512 * h2 + w],
                                        msk[:, j, 0:w],
                                    )
                            for h2 in range(2):
                                nc.tensor.matmul(
                                    ps_y[h2],
                                    lhsT=v_sb[:, kb, 2 * hp + h2, :],
                                    rhs=att[:, 512 * h2 : 512 * (h2 + 1)],
                                    start=(kb == 0), stop=(kb == nkb - 1),
                                    skip_group_check=True,
                                )
                        for h2 in range(2):
                            rec = att_pool.tile([HEAD, 512], F32, tag="rec", bufs=2)
                            nc.vector.reciprocal(
                                rec, ps_y[h2][HEAD : 2 * HEAD, :]
                            )
                            nc.vector.tensor_mul(
                                yTq[64 * h2 : 64 * (h2 + 1), hp, :],
                                ps_y[h2][0:HEAD, :], rec,
                            )

                    def emit_ag(qc, yTq):
                        nc.sync.dma_start(
                            out=y_bounce[qc].rearrange("(c p) n -> p c n", p=128),
                            in_=yTq,
                        )
                        nc.gpsimd.collective_compute(
                            "AllGather",
                            AluOpType.bypass,
                            replica_groups=GROUPS,
                            ins=[y_bounce[qc][:]],
                            outs=[ag_bounce[qc][:]],
                        )

                    def drain(gen):
                        if gen is None:
                            return
                        try:
                            next(gen)
                        except StopIteration:
                            pass

                    gen0 = None
                    for qc in range(4):
                        yTq = yT_pool.tile([128, 3, 512], F8, tag="yT", name=f"yT{qc}")
                        for hp in range(3):
                            attn_hp(qc, hp, yTq)
                            if hp == 2:
                                emit_ag(qc, yTq)
                            if qc >= 2:
                                drain(gen0)
                        if qc == 1:
                            gen0 = ffn_block(0)
                            drain(gen0)  # issue block-0 gather/residual DMAs

                    gen1 = ffn_block(1)
                    for _ in range(8):
                        drain(gen1)

    _split_ctrl_waits(nc)
    return nc


_NC_CACHE = None


def _get_nc():
    global _NC_CACHE
    if _NC_CACHE is None:
        _NC_CACHE = build_nc()
    return _NC_CACHE


def _pack_pair(w, f8):
    """[K, M] -> [128, K//256, 2, M] fp8, rows (256a + 128j + p) -> [p, a, j]."""
    K, M = w.shape
    a = np.clip(w * WS, -240.0, 240.0).astype(f8)
    return np.ascontiguousarray(
        a.reshape(K // 256, 2, 128, M).transpose(2, 0, 1, 3)
    )


def _prep_inputs(x, ln1_g, ln1_b, w_attn, b_attn, w_aproj, b_aproj,
                 ln2_g, ln2_b, w_fc, b_fc, w_mproj, b_mproj):
    bf = ml_dtypes.bfloat16
    f32 = np.float32
    f8 = mybir.dt.np(F8)
    x = np.asarray(x, f32)
    ln1_g = np.asarray(ln1_g, f32); ln1_b = np.asarray(ln1_b, f32)
    ln2_g = np.asarray(ln2_g, f32); ln2_b = np.asarray(ln2_b, f32)
    w_attn = np.asarray(w_attn, f32); b_attn = np.asarray(b_attn, f32)
    w_aproj = np.asarray(w_aproj, f32); b_aproj = np.asarray(b_aproj, f32)
    w_fc = np.asarray(w_fc, f32); b_fc = np.asarray(b_fc, f32)
    w_mproj = np.asarray(w_mproj, f32); b_mproj = np.asarray(b_mproj, f32)

    # fold ln1 gain into w_attn rows; ln1 bias into b_attn
    w_attn_f = ln1_g[:, None] * w_attn
    b_attn_f = b_attn + ln1_b @ w_attn
    wq = w_attn_f[:, 0:N_EMBED]; bq = b_attn_f[0:N_EMBED]
    wk = w_attn_f[:, N_EMBED : 2 * N_EMBED]; bk = b_attn_f[N_EMBED : 2 * N_EMBED]
    wv = w_attn_f[:, 2 * N_EMBED :]; bv = b_attn_f[2 * N_EMBED :]

    w_fc_f = ln2_g[:, None] * w_fc
    b_fc_f = b_fc + ln2_b @ w_fc

    # causal diagonal masks in transposed layout: msk[k, j, q] = k + 128j <= q
    kk = np.arange(128)[:, None, None]
    jj = np.arange(4)[None, :, None]
    qq = np.arange(512)[None, None, :]
    msk = ((kk + 128 * jj) <= qq).astype(bf)

    wap_p = _pack_pair(w_aproj, f8)
    wfc_p = _pack_pair(w_fc_f, f8)
    wmp_p = _pack_pair(w_mproj, f8)
    bfc_t = np.ascontiguousarray(b_fc_f.reshape(24, 128).T).astype(f32)
    bmp64 = (WS * b_mproj)[None, :].astype(bf)

    per_rank = []
    for r in range(2):
        hsel = slice(r * DHG, (r + 1) * DHG)
        bqk = np.zeros((128, 6), f32)
        for m in range(3):
            bqk[:, m] = bq[hsel][128 * m : 128 * (m + 1)]
            bqk[:, 3 + m] = bk[hsel][128 * m : 128 * (m + 1)]
        per_rank.append(
            dict(
                wq=_pack_pair(wq[:, hsel], f8),
                wk=_pack_pair(wk[:, hsel], f8),
                wv=_pack_pair(wv[:, hsel], f8),
                bqk=bqk,
                bv=(WS * np.ascontiguousarray(bv[hsel]))[None, :].astype(bf),
                wap=wap_p,
                wfc=wfc_p,
                bfc=bfc_t,
                wmp=wmp_p,
                bmp=bmp64,
                msk=msk,
            )
        )

    in_maps = []
    for c in range(8):
        b_i, r = c // 2, c % 2
        m = dict(per_rank[r])
        m["x"] = np.ascontiguousarray(x[b_i])
        m["xb"] = np.ascontiguousarray(x[b_i] + b_aproj[None, :])
        in_maps.append(m)
    return in_maps


def kernel(**inputs):
    nc = _get_nc()
    in_maps = _prep_inputs(**inputs)
    res = run_bass_kernel_spmd(nc, in_maps, list(range(8)))
    out = np.empty((B, T, N_EMBED), np.float32)
    for c in range(8):
        b_i, r = c // 2, c % 2
        o = res.results[c]["out"]
        out[b_i, 512 * r : 512 * (r + 1), :] = o[0:512]
        out[b_i, 1024 + 512 * r : 1024 + 512 * (r + 1), :] = o[512:1024]
    return out


# revision 18
# speedup vs baseline: 1.1267x; 1.0040x over previous
"""GPT-2 transformer block on 8 Trainium2 NeuronCores.

Sharding: core c = (batch b = c//2, rank r = c%2).  Pairs (2b, 2b+1) share a
batch: each core computes ln1 + qkv for its 6 of 12 heads over the full
sequence (T=2048), causal attention in transposed layout, an intra-pair
AllGather of per-head outputs per 512-token quarter, then token-parallel
aproj + ln2 + FFN where rank r owns quarters {r, r+2}.  The FFN work for the
first owned quarter is interleaved into the scalar-bound (exp) attention
stream of quarters 2-3 so the PE array never idles.  Dense matmuls (qkv,
aproj, fc, mproj) run in fp8-e4m3 DoubleRow (pairs of adjacent 128-chunks
along the contraction dim); attention scores/AV stay bf16.  Weights are
pre-scaled by 64 before the fp8 cast (avoids the subnormal range) and the
1/64 is folded into the consuming activation/vector op; the attention score
scale 1/sqrt(64) is folded into the exp activation's scale.  LayerNorm
gains/biases and b_aproj are folded into weights / the residual input on the
host.
"""

import numpy as np
import ml_dtypes

import concourse.bass as bass
import concourse.tile as tile
from concourse import mybir
from concourse.alu_op_type import AluOpType
from concourse.masks import make_identity
from concourse.bass_utils import run_bass_kernel_spmd

BF16 = mybir.dt.bfloat16
F32 = mybir.dt.float32
F8 = mybir.dt.float8e4
AF = mybir.ActivationFunctionType
DR = mybir.MatmulPerfMode.DoubleRow
MUL = AluOpType.mult
ADD = AluOpType.add

N_EMBED = 768
N_HEAD = 12
HEAD = 64
B, T = 4, 2048
D4 = 4 * N_EMBED          # 3072
HG = N_HEAD // 2          # heads per core = 6
DHG = HG * HEAD           # 384
TOWN = T // 2             # own tokens per core = 1024
GROUPS = [[2 * i, 2 * i + 1] for i in range(4)]
EPS = 1e-5
WS = 64.0                 # fp8 weight pre-scale
IWS = 1.0 / WS

# walrus single-wait-per-instruction limit workaround ------------------------


def _split_ctrl_waits(nc, max_waits=1):
    fn = nc.m.functions[0]
    for bb in fn.blocks:
        insts = list(bb.instructions)
        changed = False
        new_list = []
        for inst in insts:
            si = inst.sync_info
            waits = list(si.on_wait) if (si is not None and si.on_wait) else []
            if len(waits) > max_waits:
                keep = waits[-max_waits:]
                extra = waits[:-max_waits]
                k = 0
                while extra:
                    batch, extra = extra[:max_waits], extra[max_waits:]
                    nop = mybir.InstNoOp(name=f"{inst.name}_wsplit{k}", ins=[], outs=[])
                    nop.engine = inst.engine
                    nop.sync_info = mybir.SyncInfo(on_wait=batch, on_update=[])
                    new_list.append(nop)
                    k += 1
                inst.sync_info = mybir.SyncInfo(
                    on_wait=keep, on_update=list(si.on_update) if si.on_update else []
                )
                changed = True
            new_list.append(inst)
        if changed:
            bb.instructions = new_list


# ---------------------------------------------------------------------------


def _ln_stats4(nc, pools, x_aps):
    """Batched LN stats over 4 [128,768] f32 tiles -> (r, nmr) [128,4] f32."""
    small = pools["small"]
    n = len(x_aps)
    stats = small.tile([128, n, 3, 6], F32, tag="stats")
    for t, x in enumerate(x_aps):
        xv = x.rearrange("p (s d) -> p s d", s=3)
        for s in range(3):
            nc.vector.bn_stats(stats[:, t, s, :], xv[:, s, :])
    mv = small.tile([128, n, 2], F32, tag="mv")
    for t in range(n):
        nc.vector.bn_aggr(mv[:, t, :], stats[:, t, :, :])
    sd = small.tile([128, n], F32, tag="sd")
    nc.scalar.activation(sd, mv[:, :, 1], AF.Sqrt, bias=pools["eps"], scale=1.0)
    r = small.tile([128, n], F32, tag="r")
    nc.vector.reciprocal(r, sd)
    nmr = small.tile([128, n], F32, tag="nmr")
    nc.vector.scalar_tensor_tensor(nmr, mv[:, :, 0], -1.0, r, op0=MUL, op1=MUL)
    return r, nmr


def _ln_apply_transpose(nc, pools, x_ap, r1, nmr1, dstT, tcol, on_vector):
    """Normalize x [128,768] -> fp8, PE-transpose into dstT[:, c, tcol:+128]."""
    ln_t = pools["lnp"].tile([128, N_EMBED], BF16, tag="ln_t")
    nc.vector.tensor_scalar(ln_t, x_ap, r1, nmr1, op0=MUL, op1=ADD)
    psf = pools["psum_f"]
    ps4 = psf.tile([128, 512], BF16, tag="ps", name="tp4")
    for c in range(4):
        nc.tensor.transpose(
            ps4[:, 128 * c : 128 * (c + 1)], ln_t[:, 128 * c : 128 * (c + 1)],
            pools["ident"],
        )
    ps2 = psf.tile([128, 512], BF16, tag="ps", name="tp2")
    for c in range(2):
        nc.tensor.transpose(
            ps2[:, 128 * c : 128 * (c + 1)], ln_t[:, 512 + 128 * c : 512 + 128 * (c + 1)],
            pools["ident"],
        )
    src4 = ps4.rearrange("p (c x) -> p c x", c=4)
    src2 = ps2[:, 0:256].rearrange("p (c x) -> p c x", c=2)
    if on_vector:
        nc.vector.tensor_copy(dstT[:, 0:4, tcol : tcol + 128], src4)
        nc.vector.tensor_copy(dstT[:, 4:6, tcol : tcol + 128], src2)
    else:
        nc.scalar.copy(dstT[:, 0:4, tcol : tcol + 128], src4)
        nc.scalar.copy(dstT[:, 4:6, tcol : tcol + 128], src2)


def build_nc():
    nc = bass.Bass()

    x_ext = nc.declare_dram_parameter("x", [T, N_EMBED], BF16, isOutput=False)
    xb_ext = nc.declare_dram_parameter("xb", [T, N_EMBED], F32, isOutput=False)
    wq_ext = nc.declare_dram_parameter("wq", [128, 3, 2, DHG], F8, isOutput=False)
    wk_ext = nc.declare_dram_parameter("wk", [128, 3, 2, DHG], F8, isOutput=False)
    wv_ext = nc.declare_dram_parameter("wv", [128, 3, 2, DHG], F8, isOutput=False)
    bqk_ext = nc.declare_dram_parameter("bqk", [128, 6], F32, isOutput=False)
    bv_ext = nc.declare_dram_parameter("bv", [1, DHG], BF16, isOutput=False)
    wap_ext = nc.declare_dram_parameter("wap", [128, 3, 2, N_EMBED], F8, isOutput=False)
    wfc_ext = nc.declare_dram_parameter("wfc", [128, 3, 2, D4], F8, isOutput=False)
    bfc_ext = nc.declare_dram_parameter("bfc", [128, 24], F32, isOutput=False)
    wmp_ext = nc.declare_dram_parameter("wmp", [128, 12, 2, N_EMBED], F8, isOutput=False)
    bmp_ext = nc.declare_dram_parameter("bmp", [1, N_EMBED], BF16, isOutput=False)
    msk_ext = nc.declare_dram_parameter("msk", [128, 4, 512], BF16, isOutput=False)
    out_ext = nc.declare_dram_parameter("out", [TOWN, N_EMBED], F32, isOutput=True)

    y_bounce = nc.dram_tensor("y_bounce", [4, DHG, 512], BF16)
    ag_bounce = nc.dram_tensor("ag_bounce", [4, 2 * DHG, 512], BF16)

    with tile.TileContext(nc) as tc:
        with (
            tc.tile_pool(name="perm", bufs=1) as perm,
            tc.tile_pool(name="small", bufs=3) as small,
            tc.tile_pool(name="psum_s", bufs=2, space="PSUM") as psum_s,
            tc.tile_pool(name="psum_y", bufs=2, space="PSUM") as psum_y,
            tc.tile_pool(name="psum_f", bufs=2, space="PSUM") as psum_f,
            tc.tile_pool(name="lnp", bufs=3) as lnp,
        ):
            # rank within the pair, for dynamic token-quarter addressing
            rank_reg = nc.gpsimd.alloc_register()
            nc.gpsimd.cc_rank_ld(rank_reg, replica_groups=GROUPS)
            rank = nc.gpsimd.snap(rank_reg, donate=True)

            ident = perm.tile([128, 128], BF16, tag="ident")
            make_identity(nc, ident)
            eps_t = perm.tile([128, 1], F32, tag="eps")
            nc.vector.memset(eps_t, EPS)
            ones_row = perm.tile([1, 128], BF16, tag="ones_row")
            nc.vector.memset(ones_row, 1.0)

            # PE warm-up: keep the array busy while the first DMAs land so
            # the HAM clock gate opens before the real matmuls start.
            for wix in range(10):
                pw = psum_s.tile([128, 1024], F32, tag="ps2", name=f"warm{wix}")
                nc.tensor.matmul(pw[:, 0:128], lhsT=ident, rhs=ident, start=True, stop=True)

            msk = perm.tile([128, 4, 512], BF16, tag="msk")
            nc.gpsimd.dma_start(out=msk, in_=msk_ext[:, :, :])
            wq_sb = perm.tile([128, 3, 2, DHG], F8, tag="wq")
            nc.gpsimd.dma_start(out=wq_sb, in_=wq_ext[:, :, :, :])
            wk_sb = perm.tile([128, 3, 2, DHG], F8, tag="wk")
            nc.gpsimd.dma_start(out=wk_sb, in_=wk_ext[:, :, :, :])
            wv_sb = perm.tile([128, 3, 2, DHG], F8, tag="wv")
            nc.gpsimd.dma_start(out=wv_sb, in_=wv_ext[:, :, :, :])
            bqk_sb = perm.tile([128, 6], F32, tag="bqk")
            nc.gpsimd.dma_start(out=bqk_sb, in_=bqk_ext[:, :])
            bv_sb = perm.tile([1, DHG], BF16, tag="bv")
            nc.gpsimd.dma_start(out=bv_sb, in_=bv_ext[:, :])
            wap_sb = perm.tile([128, 3, 2, N_EMBED], F8, tag="wap")
            nc.gpsimd.dma_start(out=wap_sb, in_=wap_ext[:, :, :, :])
            wfc_sb = perm.tile([128, 3, 2, D4], F8, tag="wfc")
            nc.gpsimd.dma_start(out=wfc_sb, in_=wfc_ext[:, :, :, :])
            bfc_sb = perm.tile([128, 24], F32, tag="bfc")
            nc.gpsimd.dma_start(out=bfc_sb, in_=bfc_ext[:, :])
            wmp_sb = perm.tile([128, 12, 2, N_EMBED], F8, tag="wmp")
            nc.gpsimd.dma_start(out=wmp_sb, in_=wmp_ext[:, :, :, :])
            bmp_sb = perm.tile([1, N_EMBED], BF16, tag="bmp")
            nc.gpsimd.dma_start(out=bmp_sb, in_=bmp_ext[:, :])

            pools = {
                "small": small, "psum_f": psum_f, "lnp": lnp,
                "ident": ident, "eps": eps_t,
            }

            with tc.tile_pool(name="qkv", bufs=1) as qkv_pool:
                qT = qkv_pool.tile([128, 3, T], BF16, tag="qT")
                kT = qkv_pool.tile([128, 3, T], BF16, tag="kT")
                v_sb = qkv_pool.tile([128, 16, HG, 2 * HEAD], BF16, tag="v_sb")
                ln1xT = qkv_pool.tile([128, 6, T], F8, tag="ln1xT")

                def qk_group(g):
                    for dst, w_sb, bcol in ((qT, wq_sb, 0), (kT, wk_sb, 3)):
                        for m in range(3):
                            ps = psum_f.tile([128, 512], F32, tag="ps", name=f"qk{g}{m}")
                            for a in range(3):
                                nc.tensor.matmul(
                                    ps,
                                    lhsT=w_sb[:, a, :, 128 * m : 128 * (m + 1)],
                                    rhs=ln1xT[:, 2 * a : 2 * a + 2, 512 * g : 512 * (g + 1)],
                                    start=(a == 0), stop=(a == 2), perf_mode=DR,
                                )
                            nc.scalar.activation(
                                dst[:, m, 512 * g : 512 * (g + 1)], ps, AF.Identity,
                                bias=bqk_sb[:, bcol + m : bcol + m + 1], scale=IWS,
                            )

                def v_group(g):
                    for tl in range(4):
                        t = 4 * g + tl
                        ps = psum_f.tile([128, 512], F32, tag="ps", name=f"v{t}")
                        for a in range(3):
                            nc.tensor.matmul(
                                ps[:, 0:DHG],
                                lhsT=ln1xT[:, 2 * a : 2 * a + 2, 128 * t : 128 * (t + 1)],
                                rhs=wv_sb[:, a, :, :],
                                start=(a == 0), stop=False, perf_mode=DR,
                            )
                        nc.tensor.matmul(
                            ps[:, 0:DHG], lhsT=ones_row, rhs=bv_sb,
                            start=False, stop=True, skip_group_check=True,
                        )
                        nc.vector.tensor_scalar(
                            v_sb[:, t, :, 0:HEAD],
                            ps[:, 0:DHG].rearrange("p (h d) -> p h d", h=HG),
                            IWS, None, op0=MUL,
                        )

                # ===== phase A: ln1 + transpose over full T =====
                with tc.tile_pool(name="xpool", bufs=8) as xpool:
                    for g in range(4):
                        xts = []
                        for tl in range(4):
                            t = 4 * g + tl
                            x_t = xpool.tile([128, N_EMBED], BF16, tag="x_t", name=f"x{t}")
                            nc.sync.dma_start(out=x_t, in_=x_ext[128 * t : 128 * (t + 1), :])
                            xts.append(x_t)
                        r4, nmr4 = _ln_stats4(nc, pools, xts)
                        for tl in range(4):
                            _ln_apply_transpose(
                                nc, pools, xts[tl], r4[:, tl : tl + 1],
                                nmr4[:, tl : tl + 1], ln1xT, 128 * (4 * g + tl),
                                on_vector=False,
                            )
                    # QK/V for the first two quarters; the rest is deferred into
                    # the attention stream to keep the PE busy there.
                    qk_group(0)
                    qk_group(1)
                    v_group(0)
                    v_group(1)
                    nc.vector.memset(v_sb[:, :, :, HEAD : 2 * HEAD], 1.0)

                # ===== phase C: attention + interleaved FFN =====
                with (
                    tc.tile_pool(name="attp", bufs=4) as att_pool,
                    tc.tile_pool(name="yTp", bufs=2) as yT_pool,
                    tc.tile_pool(name="yfp", bufs=2) as yf_pool,
                    tc.tile_pool(name="xbp", bufs=4) as xb_pool,
                    tc.tile_pool(name="x1p", bufs=1) as x1_pool,
                    tc.tile_pool(name="ln2p", bufs=2) as ln2_pool,
                    tc.tile_pool(name="hTp", bufs=1) as hT_pool,
                    tc.tile_pool(name="outp", bufs=3) as outp,
                ):
                    xb_view = xb_ext.rearrange(
                        "(ii hh n p) d -> p ii hh n d", ii=2, hh=2, n=4, p=128
                    )
                    ag_v = ag_bounce.rearrange(
                        "(ii hh) (c p) n -> p ii hh c n", ii=2, hh=2, p=128
                    )

                    def ffn_block(i):
                        # processes token quarter (2*i + rank): 4 tiles of 128
                        yfT = yf_pool.tile([128, HG, 512], BF16, tag="yf", name=f"yf{i}")
                        nc.gpsimd.dma_start(
                            out=yfT, in_=ag_v[:, i, bass.ds(rank, 1), :, :]
                        )
                        xbts = []
                        for tt in range(4):
                            xbt = xb_pool.tile(
                                [128, N_EMBED], F32, tag="xb", name=f"xb{i}{tt}"
                            )
                            nc.gpsimd.dma_start(
                                out=xbt, in_=xb_view[:, i, bass.ds(rank, 1), tt, :]
                            )
                            xbts.append(xbt)
                        yf8 = yf_pool.tile([128, HG, 512], F8, tag="yf8", name=f"yf8{i}")
                        nc.vector.tensor_copy(yf8, yfT)
                        yield  # DMAs issued

                        x1 = x1_pool.tile([128, 4, N_EMBED], F32, tag="x1", name=f"x1_{i}")
                        for tt in range(4):
                            for n0, n1 in ((0, 512), (512, 768)):
                                w = n1 - n0
                                ps = psum_f.tile(
                                    [128, 512], F32, tag="ps", name=f"ap{i}{tt}{n0}"
                                )
                                for a in range(3):
                                    nc.tensor.matmul(
                                        ps[:, 0:w],
                                        lhsT=yf8[:, 2 * a : 2 * a + 2, 128 * tt : 128 * (tt + 1)],
                                        rhs=wap_sb[:, a, :, n0:n1],
                                        start=(a == 0), stop=(a == 2), perf_mode=DR,
                                    )
                                nc.vector.scalar_tensor_tensor(
                                    x1[:, tt, n0:n1], ps[:, 0:w], IWS, xbts[tt][:, n0:n1],
                                    op0=MUL, op1=ADD,
                                )
                        yield  # aproj done

                        r4, nmr4 = _ln_stats4(nc, pools, [x1[:, tt, :] for tt in range(4)])
                        ln2xT = ln2_pool.tile([128, 6, 512], F8, tag="ln2xT", name=f"l2T{i}")
                        for tt in range(4):
                            _ln_apply_transpose(
                                nc, pools, x1[:, tt, :], r4[:, tt : tt + 1],
                                nmr4[:, tt : tt + 1], ln2xT, 128 * tt, on_vector=True,
                            )
                        yield  # ln2 done

                        hT = hT_pool.tile([128, 24, 512], F8, tag="hT", name=f"hT{i}")
                        for m in range(24):
                            ps = psum_f.tile([128, 512], F32, tag="ps", name=f"fc{i}{m}")
                            for a in range(3):
                                nc.tensor.matmul(
                                    ps,
                                    lhsT=wfc_sb[:, a, :, 128 * m : 128 * (m + 1)],
                                    rhs=ln2xT[:, 2 * a : 2 * a + 2, :],
                                    start=(a == 0), stop=(a == 2), perf_mode=DR,
                                )
                            nc.scalar.activation(
                                hT[:, m, :], ps, AF.Gelu,
                                bias=bfc_sb[:, m : m + 1], scale=IWS,
                            )
                        yield  # fc done

                        for tt in range(4):
                            o_t = outp.tile([128, N_EMBED], F32, tag="o_t", name=f"o{i}{tt}")
                            for n0, n1 in ((0, 512), (512, 768)):
                                w = n1 - n0
                                ps = psum_f.tile(
                                    [128, 512], F32, tag="ps", name=f"mp{i}{tt}{n0}"
                                )
                                for a in range(12):
                                    nc.tensor.matmul(
                                        ps[:, 0:w],
                                        lhsT=hT[:, 2 * a : 2 * a + 2, 128 * tt : 128 * (tt + 1)],
                                        rhs=wmp_sb[:, a, :, n0:n1],
                                        start=(a == 0), stop=False, perf_mode=DR,
                                    )
                                nc.tensor.matmul(
                                    ps[:, 0:w], lhsT=ones_row, rhs=bmp_sb[:, n0:n1],
                                    start=False, stop=True, skip_group_check=True,
                                )
                                nc.vector.scalar_tensor_tensor(
                                    o_t[:, n0:n1], ps[:, 0:w], IWS, x1[:, tt, n0:n1],
                                    op0=MUL, op1=ADD,
                                )
                            nc.sync.dma_start(
                                out=out_ext[512 * i + 128 * tt : 512 * i + 128 * (tt + 1), :],
                                in_=o_t,
                            )
                            if tt == 1:
                                yield  # first half of mproj done

                    def attn_hp(qc, hp, yTq):
                        qoff = 512 * qc
                        nkb = 4 * (qc + 1)
                        ps_y = [
                            psum_y.tile([128, 512], F32, tag="py", name=f"py{qc}{hp}{h2}")
                            for h2 in range(2)
                        ]
                        for kb in range(nkb):
                            ps_s = psum_s.tile([128, 1024], F32, tag="ps2")
                            for h2 in range(2):
                                lo, hi = 64 * h2, 64 * (h2 + 1)
                                nc.tensor.matmul(
                                    ps_s[:, 512 * h2 : 512 * (h2 + 1)],
                                    lhsT=kT[lo:hi, hp, 128 * kb : 128 * (kb + 1)],
                                    rhs=qT[lo:hi, hp, qoff : qoff + 512],
                                    start=True, stop=True,
                                )
                            att = att_pool.tile([128, 1024], BF16, tag="att")
                            nc.scalar.activation(att, ps_s, AF.Exp, scale=0.125)
                            j = kb - 4 * qc
                            if j >= 0:
                                w = 128 * (j + 1)
                                for h2 in range(2):
                                    nc.gpsimd.tensor_mul(
                                        att[:, 512 * h2 : 512 * h2 + w],
                                        att[:, 512 * h2 : 512 * h2 + w],
                                        msk[:, j, 0:w],
                                    )
                            for h2 in range(2):
                                nc.tensor.matmul(
                                    ps_y[h2],
                                    lhsT=v_sb[:, kb, 2 * hp + h2, :],
                                    rhs=att[:, 512 * h2 : 512 * (h2 + 1)],
                                    start=(kb == 0), stop=(kb == nkb - 1),
                                    skip_group_check=True,
                                )
                        for h2 in range(2):
                            rec = att_pool.tile([HEAD, 512], F32, tag="rec", bufs=2)
                            nc.vector.reciprocal(
                                rec, ps_y[h2][HEAD : 2 * HEAD, :]
                            )
                            nc.vector.tensor_mul(
                                yTq[64 * h2 : 64 * (h2 + 1), hp, :],
                                ps_y[h2][0:HEAD, :], rec,
                            )

                    def emit_ag(qc, yTq):
                        nc.sync.dma_start(
                            out=y_bounce[qc].rearrange("(c p) n -> p c n", p=128),
                            in_=yTq,
                        )
                        nc.gpsimd.collective_compute(
                            "AllGather",
                            AluOpType.bypass,
                            replica_groups=GROUPS,
                            ins=[y_bounce[qc][:]],
                            outs=[ag_bounce[qc][:]],
                        )

                    def drain(gen):
                        if gen is None:
                            return
                        try:
                            next(gen)
                        except StopIteration:
                            pass

                    gen0 = None
                    for qc in range(4):
                        yTq = yT_pool.tile([128, 3, 512], BF16, tag="yT", name=f"yT{qc}")
                        for hp in range(3):
                            attn_hp(qc, hp, yTq)
                            if hp == 2:
                                emit_ag(qc, yTq)
                            if qc >= 2:
                                drain(gen0)
                            elif qc == 0 and hp == 0:
                                qk_group(2)
                            elif qc == 0 and hp == 1:
                                v_group(2)
                            elif qc == 1 and hp == 0:
                                qk_group(3)
                            elif qc == 1 and hp == 1:
                                v_group(3)
                        if qc == 1:
                            gen0 = ffn_block(0)
                            drain(gen0)  # issue block-0 gather/residual DMAs

                    gen1 = ffn_block(1)
                    for _ in range(8):
                        drain(gen1)

    _split_ctrl_waits(nc)
    return nc


_NC_CACHE = None


def _get_nc():
    global _NC_CACHE
    if _NC_CACHE is None:
        _NC_CACHE = build_nc()
    return _NC_CACHE


def _pack_pair(w, f8):
    """[K, M] -> [128, K//256, 2, M] fp8, rows (256a + 128j + p) -> [p, a, j]."""
    K, M = w.shape
    a = np.clip(w * WS, -240.0, 240.0).astype(f8)
    return np.ascontiguousarray(
        a.reshape(K // 256, 2, 128, M).transpose(2, 0, 1, 3)
    )


def _prep_inputs(x, ln1_g, ln1_b, w_attn, b_attn, w_aproj, b_aproj,
                 ln2_g, ln2_b, w_fc, b_fc, w_mproj, b_mproj):
    bf = ml_dtypes.bfloat16
    f32 = np.float32
    f8 = mybir.dt.np(F8)
    x = np.asarray(x, f32)
    ln1_g = np.asarray(ln1_g, f32); ln1_b = np.asarray(ln1_b, f32)
    ln2_g = np.asarray(ln2_g, f32); ln2_b = np.asarray(ln2_b, f32)
    w_attn = np.asarray(w_attn, f32); b_attn = np.asarray(b_attn, f32)
    w_aproj = np.asarray(w_aproj, f32); b_aproj = np.asarray(b_aproj, f32)
    w_fc = np.asarray(w_fc, f32); b_fc = np.asarray(b_fc, f32)
    w_mproj = np.asarray(w_mproj, f32); b_mproj = np.asarray(b_mproj, f32)

    # fold ln1 gain into w_attn rows; ln1 bias into b_attn
    w_attn_f = ln1_g[:, None] * w_attn
    b_attn_f = b_attn + ln1_b @ w_attn
    wq = w_attn_f[:, 0:N_EMBED]; bq = b_attn_f[0:N_EMBED]
    wk = w_attn_f[:, N_EMBED : 2 * N_EMBED]; bk = b_attn_f[N_EMBED : 2 * N_EMBED]
    wv = w_attn_f[:, 2 * N_EMBED :]; bv = b_attn_f[2 * N_EMBED :]

    w_fc_f = ln2_g[:, None] * w_fc
    b_fc_f = b_fc + ln2_b @ w_fc

    # causal diagonal masks in transposed layout: msk[k, j, q] = k + 128j <= q
    kk = np.arange(128)[:, None, None]
    jj = np.arange(4)[None, :, None]
    qq = np.arange(512)[None, None, :]
    msk = ((kk + 128 * jj) <= qq).astype(bf)

    wap_p = _pack_pair(w_aproj, f8)
    wfc_p = _pack_pair(w_fc_f, f8)
    wmp_p = _pack_pair(w_mproj, f8)
    bfc_t = np.ascontiguousarray(b_fc_f.reshape(24, 128).T).astype(f32)
    bmp64 = (WS * b_mproj)[None, :].astype(bf)

    per_rank = []
    for r in range(2):
        hsel = slice(r * DHG, (r + 1) * DHG)
        bqk = np.zeros((128, 6), f32)
        for m in range(3):
            bqk[:, m] = bq[hsel][128 * m : 128 * (m + 1)]
            bqk[:, 3 + m] = bk[hsel][128 * m : 128 * (m + 1)]
        per_rank.append(
            dict(
                wq=_pack_pair(wq[:, hsel], f8),
                wk=_pack_pair(wk[:, hsel], f8),
                wv=_pack_pair(wv[:, hsel], f8),
                bqk=bqk,
                bv=(WS * np.ascontiguousarray(bv[hsel]))[None, :].astype(bf),
                wap=wap_p,
                wfc=wfc_p,
                bfc=bfc_t,
                wmp=wmp_p,
                bmp=bmp64,
                msk=msk,
            )
        )

    in_maps = []
    for c in range(8):
        b_i, r = c // 2, c % 2
        m = dict(per_rank[r])
        m["x"] = np.ascontiguousarray(x[b_i]).astype(bf)
        m["xb"] = np.ascontiguousarray(x[b_i] + b_aproj[None, :])
        in_maps.append(m)
    return in_maps


def kernel(**inputs):
    nc = _get_nc()
    in_maps = _prep_inputs(**inputs)
    res = run_bass_kernel_spmd(nc, in_maps, list(range(8)))
    out = np.empty((B, T, N_EMBED), np.float32)
    for c in range(8):
        b_i, r = c // 2, c % 2
        o = res.results[c]["out"]
        out[b_i, 512 * r : 512 * (r + 1), :] = o[0:512]
        out[b_i, 1024 + 512 * r : 1024 + 512 * (r + 1), :] = o[512:1024]
    return out
